# revision 1
# baseline (speedup 1.0000x reference)
"""AugAttention Trainium2 kernel.

Computes, per batch element (one NeuronCore each, data-parallel over B=8):
  xc = relu(conv1x1(x; Wc, bc))
  q = conv(conv(xc, Wq1), Wq2); k likewise; v likewise   (fused on HOST into
      one 512x512 weight + bias per branch)
  s = q^T k  (raw scores; softmax/ranking consume s * 1/sqrt(C))
  attn = softmax(s * scale)
  ranks = descending rank of s per row (double argsort)
  mask  = (rank+1)^3 for s >= 0 else 1
  out   = (attn * mask) @ v + xc

Ranking strategy: per row, bucketize s into 8190 buckets over the positive
range (all negatives collapse to bucket 1; masks of negatives don't depend
on their rank), pack = bucket*2048 + column_index (exact in fp32 up to
2^24), bitonic-sort each row's 2048-padded pack vector descending on the
Vector engine, recover the original column of each sorted position from the
low 11 bits, and scatter sorted position t (= rank) back to the original
column with GPSIMD local_scatter.  All matmuls run in fp32 on the PE.

I/O strategy (the axon tunnel runs at ~80 MB/s up / ~40 MB/s down, so
transfer bytes dominate wall-clock): x ships as fp16 [C, N] per core;
fused weights+biases ship as ONE fp16 [C, 4C+4] copy sharded over the
cores and replicated on-device by an XLA all-gather (separate jit — the
bass_exec hook only allows parameter operands); iota/identity/pad
constants are generated on device; the output returns in fp16 scaled by
2^-16 (values reach ~1e8, beyond fp16 range) and is rescaled on host.
The 8 cores are driven as two pipelined groups of 4 so group 1's upload
and execution overlap group 0's readback (the tunnel is full-duplex when
driven from separate threads).  PJRT executables are jitted once and
cached; the NEFF writes every output element, so the output-operand slot
is fed by a cached device-resident dummy instead of uploading zeros each
call.
"""
import os
import numpy as np

B, C, H, W = 8, 512, 40, 40
N = H * W            # 1600
NP = 1664            # padded to 13*128
NCH = NP // 128      # 13 chunks of 128 attention rows
NSORT = 2048
SCALE = 1.0 / float(np.sqrt(C))
OSCALE = 2.0 ** -16  # output prescale so fp16 holds |out| up to ~1e8

_cache = {}


def _sort_stages(n):
    ks = []
    k = 2
    while k <= n:
        j = k // 2
        while j >= 1:
            ks.append((k, j))
            j //= 2
        k *= 2
    return ks


def _build():
    import concourse.bass as bass
    import concourse.mybir as mybir
    import concourse.tile as tile
    from concourse import bacc

    fp32 = mybir.dt.float32
    fp16 = mybir.dt.float16
    i32 = mybir.dt.int32
    u16 = mybir.dt.uint16
    i16 = mybir.dt.int16
    A = mybir.AluOpType
    AF = mybir.ActivationFunctionType
    AX = mybir.AxisListType

    nc = bacc.Bacc("TRN2", target_bir_lowering=False, debug=False)

    xin = nc.declare_dram_parameter("xin", [C, N], fp16, isOutput=False)
    wgt = nc.declare_dram_parameter("wgt", [C, 4 * C + 4], fp16,
                                    isOutput=False)
    out_d = nc.declare_dram_parameter("out", [C, N], fp16, isOutput=True)
    s_dram = nc.dram_tensor("s_scratch", [NCH, 128, NP], fp32)

    with tile.TileContext(nc) as tc:
        with tc.tile_pool(name="sb", bufs=1) as sb, \
             tc.tile_pool(name="sc", bufs=1) as sc, \
             tc.tile_pool(name="ps", bufs=1, space="PSUM") as ps, \
             tc.tile_pool(name="tr", bufs=2, space="PSUM") as trp:

            # ---- constants, generated on device ----
            iota_u = sb.tile([128, NP], u16, tag="iotau")
            nc.gpsimd.iota(out=iota_u, pattern=[[1, NP]], base=1,
                           channel_multiplier=0)
            identi = sb.tile([128, 128], i32, tag="w0", name="identi")
            nc.gpsimd.iota(out=identi, pattern=[[1, 128]], base=0,
                           channel_multiplier=-1)
            ident = sb.tile([128, 128], fp32, tag="ident")
            nc.vector.tensor_scalar(out=ident, in0=identi, scalar1=0,
                                    scalar2=None, op0=A.is_equal)

            # ---- unpack fp16 inputs: x, weights, biases ----
            ball = sb.tile([128, 16], fp32, tag="ball")
            x_t = []
            wall = []
            for ct in range(4):
                ld = sb.tile([128, N], fp16, tag="ld", bufs=2, name="ld")
                nc.sync.dma_start(out=ld, in_=xin[ct * 128:(ct + 1) * 128, :])
                xt = sb.tile([128, NP], fp32, tag=f"x{ct}",
                             bufs=2 if ct == 2 else 1, name="xt")
                nc.vector.tensor_copy(xt[:, :N], ld[:, :N])
                nc.vector.memset(xt[:, N:NP], 0.0)
                x_t.append(xt)
                wld = sb.tile([128, 4 * C + 4], fp16, tag="ld", bufs=2,
                              name="wld")
                nc.sync.dma_start(out=wld,
                                  in_=wgt[ct * 128:(ct + 1) * 128, :])
                w = sb.tile([128, 4 * C], fp32, tag=f"w{ct}", name="w")
                nc.vector.tensor_copy(w, wld[:, :4 * C])
                wall.append(w)
                for wi in range(4):
                    nc.vector.tensor_copy(
                        ball[:, wi * 4 + ct:wi * 4 + ct + 1],
                        wld[:, 4 * C + wi:4 * C + wi + 1])

            def conv(src, wi, relu, dst_tags):
                dst = []
                for ot in range(4):
                    pss = [ps.tile([128, 416], fp32, tag=f"mm{c}",
                                   name=f"pss{c}") for c in range(4)]
                    for ct in range(4):
                        lhsT = wall[ct][:, wi * 512 + ot * 128:
                                        wi * 512 + (ot + 1) * 128]
                        for ch in range(4):
                            nc.tensor.matmul(
                                pss[ch], lhsT,
                                src[ct][:, ch * 416:(ch + 1) * 416],
                                start=(ct == 0), stop=(ct == 3))
                    d = sb.tile([128, NP], fp32, tag=dst_tags[ot], name="d")
                    for ch in range(4):
                        nc.scalar.activation(
                            out=d[:, ch * 416:(ch + 1) * 416], in_=pss[ch],
                            func=AF.Relu if relu else AF.Identity,
                            bias=ball[:, wi * 4 + ot:wi * 4 + ot + 1],
                            scale=1.0)
                    dst.append(d)
                return dst

            xc = conv(x_t, 0, True, [f"xc{t}" for t in range(4)])
            q = conv(xc, 1, False, [f"qq{t}" for t in range(4)])
            k = conv(xc, 2, False, [f"kk{t}" for t in range(4)])

            # s chunks: s[nchunk*128 + p, m] = sum_c q[c, n] * k[c, m]
            for i in range(NCH):
                pss = [ps.tile([128, 416], fp32, tag=f"mm{c}",
                               name=f"pss{c}") for c in range(4)]
                for ct in range(4):
                    lhsT = q[ct][:, i * 128:(i + 1) * 128]
                    for ch in range(4):
                        nc.tensor.matmul(
                            pss[ch], lhsT, k[ct][:, ch * 416:(ch + 1) * 416],
                            start=(ct == 0), stop=(ct == 3))
                st = sb.tile([128, NP], fp32, tag="sio", bufs=1, name="st")
                for ch in range(4):
                    nc.scalar.copy(st[:, ch * 416:(ch + 1) * 416], pss[ch])
                nc.gpsimd.memset(st[:, N:NP], -1e6)
                nc.sync.dma_start(out=s_dram[i], in_=st)

            # v after q/k die; reuse k's slots
            v = conv(xc, 3, False, [f"kk{t}" for t in range(4)])
            # transposed v, packed into q's (now dead) slots
            vtt = [sb.tile([128, 2048 if j < 3 else 512], fp32,
                           tag=f"qq{j}", name=f"vtt{j}") for j in range(4)]

            def vT(m):
                return vtt[m // 4][:, (m % 4) * 512:(m % 4) * 512 + 512]

            for m in range(NCH):
                for ct in range(4):
                    tp = trp.tile([128, 128], fp32, tag="tr")
                    nc.tensor.transpose(tp, v[ct][:, m * 128:(m + 1) * 128],
                                        ident)
                    nc.scalar.copy(
                        vT(m)[:, ct * 128:(ct + 1) * 128], tp)

            stages = _sort_stages(NSORT)

            def softmax_stats(st):
                mx = sc.tile([128, 1], fp32, tag="mx", bufs=3, name="mx")
                nc.vector.reduce_max(out=mx, in_=st, axis=AX.X)
                nb = sc.tile([128, 1], fp32, tag="nb", bufs=3, name="nb")
                nc.vector.tensor_scalar(out=nb, in0=mx, scalar1=-SCALE,
                                        scalar2=None, op0=A.mult)
                e = sb.tile([128, NP], fp32, tag="ld", bufs=2, name="e")
                z = sc.tile([128, 1], fp32, tag="z", bufs=3, name="z")
                nc.scalar.activation(out=e, in_=st, func=AF.Exp, bias=nb,
                                     scale=SCALE, accum_out=z)
                return mx, z

            def emit_prep_sort(i):
                st = sb.tile([128, NP], fp32, tag="x2", bufs=2, name="st")
                nc.sync.dma_start(out=st, in_=s_dram[i])
                mx, z = softmax_stats(st)
                mxc = sc.tile([128, 1], fp32, tag="mxc", bufs=3, name="mxc")
                nc.vector.tensor_scalar(out=mxc, in0=mx, scalar1=1e-30,
                                        scalar2=None, op0=A.max)
                rmx = sc.tile([128, 1], fp32, tag="rmx", bufs=3, name="rmx")
                nc.vector.reciprocal(out=rmx, in_=mxc)
                invw = sc.tile([128, 1], fp32, tag="invw", bufs=3,
                               name="invw")
                nc.vector.tensor_scalar(out=invw, in0=rmx, scalar1=8189.0,
                                        scalar2=None, op0=A.mult)
                tq = sb.tile([128, NP], fp32, tag="ld", bufs=2, name="tq")
                nc.vector.tensor_scalar(out=tq, in0=st, scalar1=invw[:, 0:1],
                                        scalar2=1.5, op0=A.mult, op1=A.add)
                ci = sb.tile([128, NP], i32, tag="x3", name="ci")
                nc.vector.tensor_scalar(out=ci, in0=tq, scalar1=1.0,
                                        scalar2=8191.0, op0=A.max, op1=A.min)
                pa = sb.tile([128, NSORT], fp32, tag="x0", name="pa")
                pb = sb.tile([128, NSORT], fp32, tag="x1", name="pb")
                nc.vector.scalar_tensor_tensor(
                    out=pa[:, :NP], in0=ci, scalar=2048.0, in1=iota_u,
                    op0=A.mult, op1=A.add)
                nc.gpsimd.memset(pa[:, NP:], -1.0)
                nc.gpsimd.memset(pb[:, NP:], -1.0)
                cur, oth = pa, pb
                for (kk, jj) in stages:
                    eng = nc.vector
                    last = (kk == NSORT and jj == 1)
                    if kk < NSORT:
                        span = NP if 2 * kk <= 128 else NSORT
                        na, nm = span // (2 * kk), kk // (2 * jj)

                        def apv(t, d, qq):
                            dims = [t.ap[0]]
                            if na > 1:
                                dims.append([2 * kk, na])
                            dims += [[2 * jj, nm], [1, jj]]
                            return bass.AP(
                                tensor=t.tensor,
                                offset=t.offset + d * kk + qq * jj,
                                ap=dims)
                        for d in (0, 1):
                            op_lo = A.max if d == 0 else A.min
                            op_hi = A.min if d == 0 else A.max
                            eng.tensor_tensor(out=apv(oth, d, 0),
                                              in0=apv(cur, d, 0),
                                              in1=apv(cur, d, 1), op=op_lo)
                            eng.tensor_tensor(out=apv(oth, d, 1),
                                              in0=apv(cur, d, 0),
                                              in1=apv(cur, d, 1), op=op_hi)
                    elif not last:
                        vc = cur.rearrange("p (m q r) -> p m q r",
                                           q=2, r=jj, m=NSORT // (2 * jj))
                        vo = oth.rearrange("p (m q r) -> p m q r",
                                           q=2, r=jj, m=NSORT // (2 * jj))
                        eng.tensor_tensor(
                            out=vo[:, :, 0, :], in0=vc[:, :, 0, :],
                            in1=vc[:, :, 1, :], op=A.max)
                        eng.tensor_tensor(
                            out=vo[:, :, 1, :], in0=vc[:, :, 0, :],
                            in1=vc[:, :, 1, :], op=A.min)
                    else:
                        # final stage restricted to the real 1664 positions
                        vc = cur[:, :NP].rearrange("p (m q) -> p m q", q=2)
                        vo = oth[:, :NP].rearrange("p (m q) -> p m q", q=2)
                        eng.tensor_tensor(out=vo[:, :, 0], in0=vc[:, :, 0],
                                          in1=vc[:, :, 1], op=A.max)
                        eng.tensor_tensor(out=vo[:, :, 1], in0=vc[:, :, 0],
                                          in1=vc[:, :, 1], op=A.min)
                    cur, oth = oth, cur
                srt = sb.tile([128, NP], fp32, tag="srt", bufs=3, name="srt")
                nc.sync.dma_start(out=srt, in_=cur[:, :NP])
                return srt, mx, z

            def emit_post(i, srt, mx, z):
                ci2 = sb.tile([128, NP], i32, tag="x3", name="ci2")
                nc.vector.tensor_copy(ci2, srt)
                nc.vector.tensor_scalar(out=ci2, in0=ci2, scalar1=2047,
                                        scalar2=None, op0=A.bitwise_and)
                idx16 = sb.tile([128, NP], i16, tag="w1", name="idx16")
                nc.vector.tensor_copy(idx16, ci2)
                rnk = sb.tile([128, NP + 2], u16, tag="w0", name="rnk")
                nc.gpsimd.local_scatter(rnk, iota_u, idx16, channels=128,
                                        num_elems=NP + 2, num_idxs=NP)
                # reload raw s
                sldp = sb.tile([128, NP], fp32, tag="w2", name="sldp")
                nc.sync.dma_start(out=sldp, in_=s_dram[i])
                pos = sb.tile([128, NP], fp32, tag="w3", name="pos")
                nc.vector.tensor_scalar(out=pos, in0=sldp, scalar1=0.0,
                                        scalar2=None, op0=A.is_ge)
                lnr = sb.tile([128, NP], fp32, tag="ld", bufs=2, name="lnr")
                nc.scalar.activation(out=lnr, in_=rnk[:, 1:NP + 1],
                                     func=AF.Ln, bias=0.0, scale=1.0)
                nc.vector.scalar_tensor_tensor(out=lnr, in0=lnr,
                                               scalar=3.0 / SCALE, in1=pos,
                                               op0=A.mult, op1=A.mult)
                nc.vector.tensor_tensor(out=lnr, in0=lnr, in1=sldp, op=A.add)
                lnz = sc.tile([128, 1], fp32, tag="lnz", bufs=3, name="lnz")
                nc.scalar.activation(out=lnz, in_=z, func=AF.Ln, bias=0.0,
                                     scale=1.0)
                ab = sc.tile([128, 1], fp32, tag="ab", bufs=3, name="ab")
                nc.vector.scalar_tensor_tensor(out=ab, in0=mx, scalar=-SCALE,
                                               in1=lnz, op0=A.mult,
                                               op1=A.subtract)
                av = pos
                nc.scalar.activation(out=av, in_=lnr, func=AF.Exp,
                                     bias=ab[:, 0:1], scale=SCALE)

                ats = sb.tile([128, NP], fp32, tag="w2", name="ats")
                for m in range(NCH):
                    tp = trp.tile([128, 128], fp32, tag="tr", name="tp")
                    nc.tensor.transpose(tp, av[:, m * 128:(m + 1) * 128],
                                        ident)
                    nc.scalar.copy(ats[:, m * 128:(m + 1) * 128], tp)
                ncols = 128 if i < NCH - 1 else 64
                for ct in range(4):
                    p4 = ps.tile([128, 128], fp32, tag=f"mm{ct}",
                                 name=f"p4_{ct}")
                    nc.tensor.matmul(
                        p4, ident, xc[ct][:, i * 128:i * 128 + 128],
                        start=True, stop=False)
                    for m in range(NCH):
                        nc.tensor.matmul(
                            p4, vT(m)[:, ct * 128:(ct + 1) * 128],
                            ats[:, m * 128:(m + 1) * 128],
                            start=False, stop=(m == NCH - 1))
                    ob = sb.tile([128, 128], fp16, tag="ob", name="ob")
                    nc.scalar.activation(out=ob, in_=p4, func=AF.Identity,
                                         bias=0.0, scale=OSCALE)
                    nc.sync.dma_start(
                        out=out_d[ct * 128:(ct + 1) * 128,
                                  i * 128:i * 128 + ncols],
                        in_=ob[:, :ncols])

            pending = []
            for i in range(NCH):
                item = emit_prep_sort(i)
                for it in pending[:]:
                    if i >= it[0] + 2:
                        emit_post(*it)
                        pending.remove(it)
                pending.append((i,) + item)
            for it in pending:
                emit_post(*it)
    nc.compile()
    return nc


def _get_nc():
    if "nc" not in _cache:
        _cache["nc"] = _build()
    return _cache["nc"]


def _pack_inputs(x, Wc, bc, Wq1, bq1, Wq2, bq2, Wk1, bk1, Wk2, bk2,
                 Wv1, bv1, Wv2, bv2):
    f = np.float32
    x = np.asarray(x, f)
    wp = np.empty((C, 4 * C + 4), np.float16)
    wp[:, 0 * C:1 * C] = np.asarray(Wc, f).T
    wp[:, 1 * C:2 * C] = np.asarray(Wq1, f).T @ np.asarray(Wq2, f).T
    wp[:, 2 * C:3 * C] = np.asarray(Wk1, f).T @ np.asarray(Wk2, f).T
    wp[:, 3 * C:4 * C] = np.asarray(Wv1, f).T @ np.asarray(Wv2, f).T
    wp[:, 4 * C + 0] = np.asarray(bc, f)
    wp[:, 4 * C + 1] = np.asarray(Wq2, f) @ np.asarray(bq1, f) + np.asarray(bq2, f)
    wp[:, 4 * C + 2] = np.asarray(Wk2, f) @ np.asarray(bk1, f) + np.asarray(bk2, f)
    wp[:, 4 * C + 3] = np.asarray(Wv2, f) @ np.asarray(bv1, f) + np.asarray(bv2, f)
    return x.reshape(B * C, N), wp


def _get_runner():
    if "run" in _cache:
        return _cache["run"]
    import jax
    import concourse.mybir as mybir
    from jax.sharding import Mesh, PartitionSpec, NamedSharding
    from jax.experimental.shard_map import shard_map
    from concourse import bass2jax
    from concourse.bass2jax import _bass_exec_p

    nc = _get_nc()
    bass2jax.install_neuronx_cc_hook()

    part_name = (nc.partition_id_tensor.name
                 if nc.partition_id_tensor else None)
    in_names, out_names, out_avals = [], [], []
    for alloc in nc.m.functions[0].allocations:
        if not isinstance(alloc, mybir.MemoryLocationSet):
            continue
        name = alloc.memorylocations[0].name
        if alloc.kind == "ExternalInput":
            if name != part_name:
                in_names.append(name)
        elif alloc.kind == "ExternalOutput":
            out_names.append(name)
            out_avals.append(jax.core.ShapedArray(
                tuple(alloc.tensor_shape), mybir.dt.np(alloc.dtype)))
    assert in_names == ["xin", "wgt"] and out_names == ["out"], (
        in_names, out_names)
    in_names_all = list(in_names) + list(out_names)
    if part_name is not None:
        in_names_all.append(part_name)
    in_names_all = tuple(in_names_all)

    def _body(*args):
        operands = list(args)
        if part_name is not None:
            operands.append(bass2jax.partition_id_tensor())
        outs = _bass_exec_p.bind(
            *operands, out_avals=tuple(out_avals), in_names=in_names_all,
            out_names=tuple(out_names), lowering_input_output_aliases=(),
            sim_require_finite=True, sim_require_nnan=True, nc=nc)
        return tuple(outs)

    import threading

    devices = jax.devices()[:B]
    assert len(devices) == B, f"need {B} devices, have {len(jax.devices())}"
    # Pipelined core groups: while group 0 executes and its output streams
    # back, group 1's input still streams up (the axon tunnel is
    # full-duplex, but only when driven from separate threads).  Measured
    # best split: [4, 4] (asymmetric [3, 5] and 4-way splits lose to
    # per-operation fixed costs).
    GSIZES = [4, 4]
    gstart = [sum(GSIZES[:g]) for g in range(len(GSIZES))]
    groups = []
    for g, gsz in enumerate(GSIZES):
        mesh = Mesh(np.asarray(devices[gstart[g]:gstart[g] + gsz]),
                    ("core",))
        shc = NamedSharding(mesh, PartitionSpec("core"))
        rep = NamedSharding(mesh, PartitionSpec())
        sharded = jax.jit(
            shard_map(_body, mesh=mesh,
                      in_specs=(PartitionSpec("core"), PartitionSpec(None),
                                PartitionSpec("core")),
                      out_specs=(PartitionSpec("core"),), check_rep=False),
            keep_unused=True)
        # The kernel writes every element of "out"; this operand's contents
        # are never read, so a device-resident dummy avoids uploading zeros.
        dummy = jax.device_put(np.zeros((gsz * C, N), np.float16), shc)
        groups.append((sharded, rep, dummy, shc))

    def run(xall, wstate):
        # Overlap group 0's fp16 cast with the caller's weight check tail
        # and this function's prologue.
        xg0_box = []
        r1_g0 = (gstart[0] + GSIZES[0]) * C
        caster = threading.Thread(
            target=lambda: xg0_box.append(
                xall[:r1_g0].astype(np.float16)))
        caster.start()
        # Device-resident weight cache: wstate["w_reps"] holds the
        # replicated on-device weights, invalidated (set to None) by
        # kernel() whenever the raw weight inputs change bit-for-bit.
        # The replicated upload costs ~0.2s but happens only on weight
        # change; steady-state calls upload no weights at all.
        if wstate.get("w_reps") is None:
            wstate["w_reps"] = [jax.device_put(wstate["wp"], grp[1])
                                for grp in groups]
        res = np.empty((B * C, N), np.float32)
        fetchers = []
        errors = []
        for g, (sharded, rep_g, dummy, shc) in enumerate(groups):
            r0, r1 = gstart[g] * C, (gstart[g] + GSIZES[g]) * C
            if g == 0:
                caster.join()
                xg = xg0_box[0]
            else:
                xg = xall[r0:r1].astype(np.float16)
            xd = jax.device_put(xg, shc)
            out = sharded(xd, wstate["w_reps"][g], dummy)[0]
            # Pre-register the D2H copy so it starts the moment the NEFF
            # finishes, instead of when the fetch thread gets scheduled.
            try:
                out._copy_to_host_async()
            except AttributeError:
                pass

            def fetch(out=out, r0=r0, r1=r1):
                try:
                    np.multiply(np.asarray(out), np.float32(1.0 / OSCALE),
                                out=res[r0:r1])
                except BaseException as e:  # noqa: BLE001
                    errors.append(e)

            th = threading.Thread(target=fetch)
            th.start()
            fetchers.append(th)
        for th in fetchers:
            th.join()
        if errors:
            raise errors[0]
        return res

    _cache["run"] = run
    return run


def kernel(x, Wc, bc, Wq1, bq1, Wq2, bq2, Wk1, bk1, Wk2, bk2, Wv1, bv1,
           Wv2, bv2):
    raw = [np.asarray(a) for a in (Wc, bc, Wq1, bq1, Wq2, bq2, Wk1, bk1,
                                   Wk2, bk2, Wv1, bv1, Wv2, bv2)]
    wstate = _cache.get("wstate")
    if wstate is None or not all(
            np.array_equal(c, a) for c, a in zip(wstate["raw"], raw)):
        _, wp = _pack_inputs(x, *raw)
        wstate = {"raw": [np.array(a) for a in raw], "wp": wp,
                  "w_reps": None}
        _cache["wstate"] = wstate
    xall = np.asarray(x, np.float32).reshape(B * C, N)
    if os.environ.get("KERNEL_SPMD"):
        # classic path (supports trace=True when the NTFF hook exists)
        from concourse.bass_utils import run_bass_kernel_spmd
        nc = _get_nc()
        xall = xall.astype(np.float16)
        in_maps = [{"xin": xall[b * C:(b + 1) * C], "wgt": wstate["wp"]}
                   for b in range(B)]
        res = run_bass_kernel_spmd(nc, in_maps, core_ids=list(range(B)),
                                   trace=bool(os.environ.get("KERNEL_TRACE")))
        kernel._last_results = res
        out16 = np.stack([res.results[b]["out"] for b in range(B)])
        out = out16.astype(np.float32) * np.float32(1.0 / OSCALE)
    else:
        out = _get_runner()(xall, wstate)
    return out.reshape(B, C, H, W)

